# revision 20
# baseline (speedup 1.0000x reference)
"""CLUB loss kernel for Trainium2 (8 NeuronCores, SPMD row-sharded).

Math: the reference returns mean_i(pos_i - neg_i), a scalar.  Both the
pos and neg terms collapse into sums that never materialize the NxN
distance matrix:

  mean_pos = -0.5/N * (A - 2B + C)
      A = sum_{i,d} x[i,d]^2 * invv[i,d]
      B = sum_{i,d} x[i,d] * mu[i,d] * invv[i,d]
      C = sum_{i,d} mu[i,d]^2 * invv[i,d]
  mean_neg = -0.5 * (S_invv . S_x2 - 2 * S_muinvv . S_x + N*C) / N^2
      S_invv = sum_i invv[i,:]     S_muinvv = sum_i mu[i,:]*invv[i,:]
      S_x    = sum_j x[j,:]        S_x2     = sum_j x[j,:]^2
  loss = mean_pos - mean_neg

Each core handles 2048 rows (2 batches of x + matching mu/logvar rows).

Layout: d-major (128, 1024): partition q = (sub-slab b, dim d), free
axis = row index.  Every reduction is a free-axis row-sum -> one fused
elementwise+accumulate instruction per quantity per chunk.

Structure (from trace analysis of the 24us baseline; ~22.1-22.8us now):
 - 3 full-width input DMAs with 4KB partition lines (doubles per-queue
   HWDGE packet throughput vs 2KB lines): SP queue carries lv then x;
   the ACT queue carries mu, behind the async act-table DMA that the
   dummy first activation hoists to kernel start (the table costs the
   ACT queue ~1.5-2us of head time; mu still beats lv+x on SP).
 - Compute at the op-count floor (13 accumulating ops):
   ACT: exp x2 (+Sinvv), square x2 (+Sx2, out feeds A), Sx full copy;
   DVE: muinvv x2 (+Smuinvv riding the accumulator), B x2, C x2, A x2.
   Pool only builds G (it shares its SBUF port with DVE and its ISA
   rejects TensorScalarPtr, so it cannot help with accumulation).
 - The 13 accumulator columns are PE-reduced against a stacked identity
   G[128,64] (G[p,d]=1 iff p%64==d) in TWO batches: the 7 early columns
   are matmul'd/copied/DMA'd while DVE still runs (hidden, and warms
   the queue); only the 6 late columns sit on the critical tail.
"""

import sys

sys.path.insert(0, "/opt/trn_rl_repo")

import numpy as np
from contextlib import ExitStack

import concourse.bass as bass
import concourse.bacc as bacc
import concourse.tile as tile
from concourse import mybir
from concourse.masks import make_identity
from concourse.bass_utils import run_bass_kernel_spmd

F32 = mybir.dt.float32
N_CORES = 8
B, D, H, W = 16, 64, 32, 32
HW = H * W                # 1024
N = B * HW                # 16384
NB = B // N_CORES         # 2 sub-slabs (batches) per core
ROWS = NB * HW            # 2048 rows per core
COLS = HW                 # free size of the (128, 1024) layout
CH = 512                  # chunk size (2 chunks)
# acc columns, ordered so the early batch (ready while compute still
# runs) occupies cols [0:NEARLY) and the late batch the rest; each
# entry is (quantity, chunk) with chunk=None for full-width.
COL_ORDER = [("Sinvv", 0), ("Sinvv", 1), ("Smuinvv", 0), ("Smuinvv", 1),
             ("Sx2", 0), ("B", 0), ("C", 0),
             ("Sx2", 1), ("Sx", None), ("B", 1), ("C", 1),
             ("A", 0), ("A", 1)]
NEARLY = 7
COLIDX = {qc: i for i, qc in enumerate(COL_ORDER)}
ACC_COLS = {"A": 2, "B": 2, "C": 2, "Sx2": 2, "Sinvv": 2, "Sx": 1,
            "Smuinvv": 2}
NCH = 2
NACC = len(COL_ORDER)     # 13


def build_nc() -> bass.Bass:
    nc = bacc.Bacc()
    # one contiguous full-width DRAM tensor per input: 4KB partition
    # lines double the per-queue HWDGE packet throughput vs 2KB
    dram = {
        nm: nc.dram_tensor(nm, [128, COLS], F32, kind="ExternalInput")
        for nm in ("xn", "mut", "lvt")
    }
    accs = nc.dram_tensor("accs", [NACC, D], F32, kind="ExternalOutput")

    with ExitStack() as ctx:
        tc = ctx.enter_context(tile.TileContext(nc))
        big = ctx.enter_context(tc.tile_pool(name="big", bufs=1))
        jp = ctx.enter_context(tc.tile_pool(name="jp", bufs=2))
        accp = ctx.enter_context(tc.tile_pool(name="accp", bufs=1))
        pp = ctx.enter_context(tc.psum_pool(name="pp", bufs=1))

        # dummy first activation: hoists the async act-table load to t~0
        dummy = big.tile([128, 1], F32)
        nc.scalar.memzero(dummy[:])

        xb = big.tile([128, COLS], F32)
        mu = big.tile([128, COLS], F32)
        lv = big.tile([128, COLS], F32)
        invv = big.tile([128, COLS], F32)
        x2 = big.tile([128, COLS], F32)
        miv = big.tile([128, COLS], F32)
        acc = accp.tile([128, NACC], F32)
        G = big.tile([128, D], F32)

        c0 = slice(0, CH)
        c1 = slice(CH, COLS)

        # ---- DMA issue: full-width tensors.  SP queue: lv then x (lv
        # gates the longest chain).  ACT queue: mu -- behind the async
        # act-table DMA, which the dummy act above hoisted to the front.
        nc.sync.dma_start(out=lv[:, :], in_=dram["lvt"][:, :])
        nc.scalar.dma_start(out=mu[:, :], in_=dram["mut"][:, :])
        nc.sync.dma_start(out=xb[:, :], in_=dram["xn"][:, :])

        def col(q, c):
            i = COLIDX[(q, c)]
            return acc[:, i:i + 1]

        M = mybir.AluOpType.mult
        EXP = mybir.ActivationFunctionType.Exp
        SQ = mybir.ActivationFunctionType.Square

        def act(q, out, in_, func, scale=1.0):
            nc.scalar.activation(
                out=out, in_=in_, func=func, bias=0.0, scale=scale,
                accum_out=q,
            )

        def stt(q, c, in0, in1, name):
            sl = c0 if c == 0 else c1
            jd = jp.tile([128, CH], F32, tag="jd", name=name)
            nc.vector.scalar_tensor_tensor(
                out=jd[:], in0=in0[:, sl], scalar=1.0, in1=in1[:, sl],
                op0=M, op1=M, accum_out=col(q, c),
            )

        # ---- ACT: exp0, exp1, sq0, sq1, Sx-copy (leaf last) ----
        act(col("Sinvv", 0), invv[:, c0], lv[:, c0], EXP, scale=-1.0)
        act(col("Sinvv", 1), invv[:, c1], lv[:, c1], EXP, scale=-1.0)
        act(col("Sx2", 0), x2[:, c0], xb[:, c0], SQ)
        act(col("Sx2", 1), x2[:, c1], xb[:, c1], SQ)
        ja = jp.tile([128, COLS], F32, tag="sxf", name="sxf")
        act(col("Sx", None), ja[:], xb[:, :],
            mybir.ActivationFunctionType.Copy)

        # ---- DVE: miv0(+Smu0), miv1(+Smu1), B0, C0, B1, C1, A0, A1 ----
        # Smuinvv rides the miv ops' accumulators for free.
        nc.vector.scalar_tensor_tensor(
            out=miv[:, c0], in0=mu[:, c0], scalar=1.0, in1=invv[:, c0],
            op0=M, op1=M, accum_out=col("Smuinvv", 0),
        )
        nc.vector.scalar_tensor_tensor(
            out=miv[:, c1], in0=mu[:, c1], scalar=1.0, in1=invv[:, c1],
            op0=M, op1=M, accum_out=col("Smuinvv", 1),
        )
        stt("B", 0, xb, miv, "b0")
        stt("C", 0, mu, miv, "jc0")
        stt("B", 1, xb, miv, "b1")
        stt("C", 1, mu, miv, "jc1")
        stt("A", 0, x2, invv, "ja0")
        stt("A", 1, x2, invv, "ja1")

        # ---- Pool: G (stacked identity) only ----
        nc.gpsimd.memset(G[:], 0.0)
        make_identity(nc, G[0:64, :], nomemset=True)
        make_identity(nc, G[64:128, :], nomemset=True)

        # ---- PE reduce + output in two batches: the early columns are
        # final by ~mid-compute, so their matmul/copy/DMA overlap the
        # remaining DVE work and warm the queue for the late batch.
        pout1 = pp.tile([NEARLY, D], F32, name="pout1")
        nc.tensor.matmul(pout1[:], acc[:, 0:NEARLY], G[:],
                         start=True, stop=True)
        sout1 = big.tile([NEARLY, D], F32)
        nc.scalar.activation(
            out=sout1[:], in_=pout1[:],
            func=mybir.ActivationFunctionType.Copy, bias=0.0, scale=1.0,
        )
        nc.sync.dma_start(out=accs[0:NEARLY, :], in_=sout1[:])

        pout2 = pp.tile([NACC - NEARLY, D], F32, name="pout2")
        nc.tensor.matmul(pout2[:], acc[:, NEARLY:NACC], G[:],
                         start=True, stop=True)
        sout2 = big.tile([NACC - NEARLY, D], F32)
        nc.scalar.activation(
            out=sout2[:], in_=pout2[:],
            func=mybir.ActivationFunctionType.Copy, bias=0.0, scale=1.0,
        )
        nc.sync.dma_start(out=accs[NEARLY:NACC, :], in_=sout2[:])
    return nc


def _ensure_ntff_hook():
    """This image's antenv lacks axon_hooks; if tracing is requested
    (e.g. BASS_TRACE=1), run_bass_kernel_spmd would die on the import.
    Register the ctypes-based hook if available, else a None hook so
    tracing degrades gracefully."""
    import types

    if "antenv.axon_hooks" in sys.modules:
        return
    try:
        import antenv.axon_hooks  # noqa: F401
        return
    except ImportError:
        pass
    hook = None
    try:
        sys.path.insert(0, "/root/.axon_site")
        from trn_agent_boot.trn_boot import _ntff_profile_via_ctypes

        hook = _ntff_profile_via_ctypes("/opt/axon/libaxon_pjrt.so")
    except Exception:
        hook = None
    mod = types.ModuleType("antenv.axon_hooks")
    mod._hook = hook
    mod.get_axon_ntff_profile_hook = lambda: mod._hook
    mod.set_axon_ntff_profile_hook = lambda h: setattr(mod, "_hook", h)
    sys.modules["antenv.axon_hooks"] = mod


_ensure_ntff_hook()

_NC = None


def _get_nc():
    global _NC
    if _NC is None:
        _NC = build_nc()
        # bacc passes legalize multi-sync-wait instructions for TRN2 codegen
        _NC.compile()
    return _NC


def make_in_maps(x, mu, logvar):
    x = np.ascontiguousarray(np.asarray(x, dtype=np.float32))
    mu = np.asarray(mu, dtype=np.float32)
    lv = np.asarray(logvar, dtype=np.float32)
    in_maps = []
    for c in range(N_CORES):
        r0 = c * ROWS
        mu_t = np.concatenate(
            [mu[r0 + b * HW:r0 + (b + 1) * HW].T for b in range(NB)], axis=0
        )
        lv_t = np.concatenate(
            [lv[r0 + b * HW:r0 + (b + 1) * HW].T for b in range(NB)], axis=0
        )
        x_t = x[c * NB:(c + 1) * NB].reshape(128, COLS)
        in_maps.append({
            "xn": np.ascontiguousarray(x_t),
            "mut": np.ascontiguousarray(mu_t),
            "lvt": np.ascontiguousarray(lv_t),
        })
    return in_maps


def combine(results) -> np.ndarray:
    # accs: [13, 64] per core; row COLIDX[(q,c)] = per-d sums (over
    # the two sub-slabs) of quantity q's chunk-c partial.
    tot = {q: np.zeros(D, dtype=np.float64) for q in ACC_COLS}
    for r in results:
        a = np.asarray(r["accs"], dtype=np.float64)  # (13, 64)
        for (q, _c), i in COLIDX.items():
            tot[q] += a[i]
    scal = {q: tot[q].sum() for q in ("A", "B", "C")}
    vec = tot
    A, Bs, C = scal["A"], scal["B"], scal["C"]
    mean_pos = -0.5 / N * (A - 2.0 * Bs + C)
    mean_D = (vec["Sinvv"] @ vec["Sx2"] - 2.0 * vec["Smuinvv"] @ vec["Sx"]
              + N * C) / float(N) ** 2
    loss = mean_pos + 0.5 * mean_D
    return np.array(loss, dtype=np.float32)


def kernel(x, mu, logvar, **_kwargs):
    nc = _get_nc()
    in_maps = make_in_maps(x, mu, logvar)
    res = run_bass_kernel_spmd(nc, in_maps, list(range(N_CORES)))
    return combine(res.results)
